# revision 26
# baseline (speedup 1.0000x reference)
"""Trainium2 Bass kernel for nn_ContactForceModel: 2-layer LSTM (B=512,T=128,D=64,H=512)
+ per-sphere decoder MLP. Data-parallel over batch across 8 NeuronCores (64 rows/core).

Strategy (per core, batch Bc=64):
- All LSTM matmuls in fp8e4m3 with MatmulPerfMode.DoubleRow: each instruction
  contracts K=256 (two 128-k-tiles packed side-by-side in the free dim) at
  0.5 cyc/col — 3.7x fewer PE cycles than the f32r formulation. Weights are
  pre-scaled x32 host-side (values ~1/sqrt(512) would underflow fp8 normals),
  x is pre-scaled x8 with Wx0 x4; every PSUM gate value is 32x the true z and
  the activations descale with their scale=1/32 input multiplier.
  Accuracy: numpy simulation of the full fp8 recurrence gives rel err ~7e-3
  vs the f64 reference (tolerance 2e-2).
- The packed-pair layout needs no data movement: the k-tile layouts
  ([128, NK*Bc] for h^T, [128, NK*G] for weights) already place k-tile pairs
  at stride Bc/G in the free dim, so a 3-dim AP [128, 2, n] views them
  directly.
- Gate nonlinearities and h are bf16 (2x Act/DVE throughput); the recurrent
  accumulator c stays f32. h^T is rebuilt each step with 4 PE transposes in
  bf16 (1.0 cyc/row) into PSUM + copies back to SBUF as fp8 (h0T on DVE,
  h1T on Act) for the next step's DoubleRow matmuls.
- PE program order per slot s: A(s)=layer0 matmuls, D(s-2)=transpose h1,
  B(s-1)=layer1 matmuls, C(s)=transpose h0, so the PE never waits on the
  cross-engine hT copies.
- PSUM budget (8 banks): zif0(2) zo0(1) zif1(2) zo1(1) zg shared(1) hTps(1).
- fp8/bf16 weights DMA straight into their SBUF tiles (no staging pool, no
  rounding copies — that dance is f32r-only); total weight traffic drops
  from 14.5MB to ~3.7MB per core.
- Decoder unchanged from the f32r version (it is a small slice of device
  time): exploits rank-1 feature structure: feat @ W0 = sphere_proj (host)
  (+) latent_proj (device) broadcast-add, pipelined in chunks across
  DVE(add) / Act+Pool(relu) / PE(z2,z3); the 16 z3 row-vector matmuls
  accumulate into distinct partitions of one [16,512] PSUM tile via a
  shifted zero-padded wout; softplus(y) = relu(y) - ln(sigmoid(|y|)) needs
  only one act-table load. The latent projection reads a one-off f32r copy
  of the final h^T (h1T itself is fp8).
- build_module(reps=N) wraps the whole body in a hardware loop for timing:
  time_kernel reports the marginal per-iteration device time, which excludes
  the ~75-85ms (drifting) axon-tunnel dispatch constant that one-shot
  wall-clock measurements are dominated by.
"""

import sys

sys.path.insert(0, "/opt/trn_rl_repo")

import numpy as np

import concourse.bass as bass
import concourse.tile as tile
from concourse import bacc, mybir
from concourse.bass_utils import run_bass_kernel_spmd
from concourse.masks import make_identity

F32 = mybir.dt.float32
F32R = mybir.dt.float32r
BF16 = mybir.dt.bfloat16
FP8 = mybir.dt.float8e4
AF = mybir.ActivationFunctionType
OP = mybir.AluOpType
DR = mybir.MatmulPerfMode.DoubleRow

N_CORES = 8
HID = 512
NK = HID // 128  # 4 k-tiles over the hidden dim
SX = 32.0  # fp8 weight pre-scale; PSUM holds SX*z, acts descale by 1/SX
SXIN = 8.0  # fp8 x pre-scale (with Wx0 x (SX/SXIN) so the x-part is SX*z too)


def pair_view(t, k2, blk, n0, n1):
    """[P, 2, n1-n0] view of k-tile pair k2 of tile t whose free dim is
    NK blocks of blk columns: element [p, i, n] = t[p, (2*k2+i)*blk + n0 + n].
    This is the packed-pair operand layout DoubleRow matmuls expect."""
    return bass.AP(
        tensor=t.tensor,
        offset=t.offset + (2 * k2) * blk + n0,
        ap=[t.ap[0], [blk, 2], [1, n1 - n0]],
    )


def build_module(T, Bc, K0h, S, use_bl1, use_b1, bout_f, reps=1):
    G = 4 * HID  # 2048
    nc = bacc.Bacc("TRN2", target_bir_lowering=False, debug=False)

    # packed fp8 inputs: xTq[p, i*T*Bc + t*Bc + b] = 8*x[t, b, i*K0h+p]
    xTq_ext = nc.declare_dram_parameter("xTq", [K0h, 2 * T * Bc], FP8, isOutput=False)
    wx0q_ext = nc.declare_dram_parameter("wx0q", [K0h, 2 * G], FP8, isOutput=False)
    wh0q_ext = nc.declare_dram_parameter("wh0q", [128, NK * G], FP8, isOutput=False)
    wx1q_ext = nc.declare_dram_parameter("wx1q", [128, NK * G], FP8, isOutput=False)
    wh1q_ext = nc.declare_dram_parameter("wh1q", [128, NK * G], FP8, isOutput=False)
    spT_ext = nc.declare_dram_parameter("spT", [128, S], F32, isOutput=False)
    w0h_ext = nc.declare_dram_parameter("w0h", [128, NK * 128], F32, isOutput=False)
    w1_ext = nc.declare_dram_parameter("w1", [128, 64], F32, isOutput=False)
    wout_ext = nc.declare_dram_parameter("wout", [64, 1], F32, isOutput=False)
    bl1_ext = nc.declare_dram_parameter("bl1", [1, G], F32, isOutput=False) if use_bl1 else None
    b1_ext = nc.declare_dram_parameter("b1", [1, 64], F32, isOutput=False) if use_b1 else None
    out_ext = nc.declare_dram_parameter("out", [Bc, S], F32, isOutput=True)

    R = Bc * S  # 8192 decoder rows per core
    NCH = R // 2048  # 4 chunks

    with tile.TileContext(nc) as tc:
      def emit_body():
        with tc.tile_pool(name="dwts", bufs=1) as dw, \
             tc.tile_pool(name="state", bufs=1) as st:

            # ---------- persistent state ----------
            c0 = st.tile([Bc, HID], F32)
            c1 = st.tile([Bc, HID], F32)
            nc.vector.memset(c0, 0.0)
            nc.vector.memset(c1, 0.0)
            # transposed hidden state (fp8), k-tile t at columns [Bc*t, Bc*t+Bc)
            h0T = st.tile([128, NK * Bc], FP8)
            h1T = st.tile([128, NK * Bc], FP8)
            nc.vector.memset(h0T.bitcast(F32), 0.0)
            nc.vector.memset(h1T.bitcast(F32), 0.0)
            # f32r copy of the final h1T for the decoder's latent projection
            h1T_dec = st.tile([128, NK * Bc], F32R)

            identf = st.tile([Bc, Bc], F32)
            make_identity(nc, identf)
            ident = st.tile([Bc, Bc], BF16)
            nc.gpsimd.tensor_copy(ident, identf)

            # ---------- weights: direct DMA (fp8 first, ordered for step 0) ----------
            wx0q = dw.tile([K0h, 2 * G], FP8)
            nc.sync.dma_start(out=wx0q, in_=wx0q_ext[:])
            xTq = dw.tile([K0h, 2 * T * Bc], FP8)
            # heads of both halves unblock the first steps, tails follow
            HD = 8 * Bc
            nc.sync.dma_start(out=xTq[:, 0:HD], in_=xTq_ext[:, 0:HD])
            nc.sync.dma_start(out=xTq[:, T * Bc:T * Bc + HD],
                              in_=xTq_ext[:, T * Bc:T * Bc + HD])
            wh0q = dw.tile([128, NK * G], FP8)
            wx1q = dw.tile([128, NK * G], FP8)
            wh1q = dw.tile([128, NK * G], FP8)
            for w, ext in ((wh0q, wh0q_ext), (wx1q, wx1q_ext), (wh1q, wh1q_ext)):
                nc.sync.dma_start(out=w[:, 0:2 * G], in_=ext[:, 0:2 * G])
            nc.sync.dma_start(out=xTq[:, HD:T * Bc], in_=xTq_ext[:, HD:T * Bc])
            nc.sync.dma_start(out=xTq[:, T * Bc + HD:], in_=xTq_ext[:, T * Bc + HD:])
            for w, ext in ((wh0q, wh0q_ext), (wx1q, wx1q_ext), (wh1q, wh1q_ext)):
                nc.sync.dma_start(out=w[:, 2 * G:], in_=ext[:, 2 * G:])

            # decoder weights (small, persistent)
            spT_sb = dw.tile([128, S], F32)
            nc.sync.dma_start(out=spT_sb, in_=spT_ext[:])
            w0h_f = dw.tile([128, NK * 128], F32)
            nc.sync.dma_start(out=w0h_f, in_=w0h_ext[:])
            w0hr = dw.tile([128, NK * 128], F32R)
            nc.vector.tensor_copy(w0hr, w0h_f)
            w1_f = dw.tile([128, 64], F32)
            nc.sync.dma_start(out=w1_f, in_=w1_ext[:])
            w1r = dw.tile([128, 64], F32R)
            nc.vector.tensor_copy(w1r, w1_f)
            wo_f = dw.tile([64, 1], F32)
            nc.sync.dma_start(out=wo_f, in_=wout_ext[:])
            # wout padded as [0*15, w, 0*15]: window [:, 15-r:31-r] puts w in
            # column r, so the 16 z3 matmuls (8 chunks x 2 j-slices) all
            # accumulate into distinct partitions of one [16, 512] PSUM tile.
            wopad_f = dw.tile([64, 31], F32)
            nc.vector.memset(wopad_f, 0.0)
            nc.vector.tensor_copy(wopad_f[:, 15:16], wo_f)
            wopad = dw.tile([64, 31], F32R)
            nc.vector.tensor_copy(wopad, wopad_f)
            ones_r = None
            if use_bl1 or use_b1:
                of = dw.tile([1, 512], F32)
                nc.vector.memset(of, 1.0)
                ones_r = dw.tile([1, 512], F32R)
                nc.vector.tensor_copy(ones_r, of)
            bl1r = None
            if use_bl1:
                # host pre-scales bl1 by SX so it lands in the scaled PSUM
                b_f = dw.tile([1, G], F32)
                nc.sync.dma_start(out=b_f, in_=bl1_ext[:])
                bl1r = dw.tile([1, G], F32R)
                nc.vector.tensor_copy(bl1r, b_f)
            b1r = None
            if use_b1:
                b_f2 = dw.tile([1, 64], F32)
                nc.sync.dma_start(out=b_f2, in_=b1_ext[:])
                b1r = dw.tile([1, 64], F32R)
                nc.vector.tensor_copy(b1r, b_f2)

            # ================= LSTM phase =================
            with tc.tile_pool(name="gates", bufs=1) as gp, \
                 tc.tile_pool(name="lpsum", bufs=1, space="PSUM") as lps:

                # PSUM: gate slices i,f contiguous per layer; o separate;
                # g shared between layers; hT transposes share one bank.
                zif = [lps.tile([Bc, 1024], F32, tag=f"zif{l}", name=f"zif{l}")
                       for l in range(2)]
                zo = [lps.tile([Bc, 512], F32, tag=f"zo{l}", name=f"zo{l}")
                      for l in range(2)]
                zg = lps.tile([Bc, 512], F32, tag="zg")
                hTps = lps.tile([128, 2 * NK * Bc], BF16, tag="hTps")

                # gate slice order in G: i(0:512) f(512:1024) g(1024:1536) o(1536:2048)
                def regions(l):
                    # (psum_region, col_slice_in_G) emitted g, i, f, o
                    return [
                        (zg, slice(1024, 1536)),
                        (zif[l][:, 0:512], slice(0, 512)),
                        (zif[l][:, 512:1024], slice(512, 1024)),
                        (zo[l], slice(1536, 2048)),
                    ]

                def mm(out, lhsT, rhs, start, stop):
                    nc.tensor.matmul(out=out, lhsT=lhsT, rhs=rhs,
                                     start=start, stop=stop, perf_mode=DR)

                def emit_A(s):
                    """Layer-0 matmuls for step s: x-terms first, then wh0."""
                    first = s == 0
                    xs = bass.AP(tensor=xTq.tensor, offset=xTq.offset + s * Bc,
                                 ap=[xTq.ap[0], [T * Bc, 2], [1, Bc]])
                    for reg, ns in regions(0):
                        mm(reg, xs, pair_view(wx0q, 0, G, ns.start, ns.stop),
                           True, first)
                    if first:
                        return
                    for reg, ns in regions(0):
                        for k2 in range(NK // 2):
                            mm(reg, pair_view(h0T, k2, Bc, 0, Bc),
                               pair_view(wh0q, k2, G, ns.start, ns.stop),
                               False, k2 == NK // 2 - 1)

                def emit_B(s):
                    """Layer-1 matmuls for step s: wx1 (reads h0T) first, then wh1."""
                    first = s == 0
                    for reg, ns in regions(1):
                        for k2 in range(NK // 2):
                            mm(reg, pair_view(h0T, k2, Bc, 0, Bc),
                               pair_view(wx1q, k2, G, ns.start, ns.stop),
                               k2 == 0,
                               first and not use_bl1 and k2 == NK // 2 - 1)
                    if use_bl1:
                        for reg, ns in regions(1):
                            nc.tensor.matmul(out=reg, lhsT=ones_r[:, 0:Bc],
                                             rhs=bl1r[:, ns], start=False,
                                             stop=first)
                    if first:
                        return
                    for reg, ns in regions(1):
                        for k2 in range(NK // 2):
                            mm(reg, pair_view(h1T, k2, Bc, 0, Bc),
                               pair_view(wh1q, k2, G, ns.start, ns.stop),
                               False, k2 == NK // 2 - 1)

                def emit_T_pe(l, h):
                    """PE-transpose h [Bc, HID] (bf16) into hTps half l."""
                    base = l * NK * Bc
                    for k in range(NK):
                        nc.tensor.transpose(
                            hTps[:, base + k * Bc:base + (k + 1) * Bc],
                            h[:, k * 128:(k + 1) * 128],
                            ident,
                        )

                def emit_acts(l, tagp):
                    """Gate nonlinearities for layer l (Act engine), descaled."""
                    gg = gp.tile([Bc, 512], BF16, tag=tagp + "gg", name="gg")
                    nc.scalar.activation(out=gg, in_=zg, func=AF.Tanh,
                                         scale=1.0 / SX)
                    gif = gp.tile([Bc, 1024], BF16, tag=tagp + "gif", name="gif")
                    nc.scalar.activation(out=gif, in_=zif[l], func=AF.Sigmoid,
                                         scale=1.0 / SX)
                    go = gp.tile([Bc, 512], BF16, tag=tagp + "go", name="go")
                    nc.scalar.activation(out=go, in_=zo[l], func=AF.Sigmoid,
                                         scale=1.0 / SX)
                    return gg, gif, go

                def emit_state(l, tagp, gg, gif, go):
                    """c update + h for layer l (DVE / Pool / Act)."""
                    c = c0 if l == 0 else c1
                    t1 = gp.tile([Bc, 512], BF16, tag=tagp + "t1", name="t1")
                    nc.vector.tensor_mul(out=t1, in0=gif[:, 0:512], in1=gg)
                    nc.gpsimd.tensor_mul(out=c, in0=gif[:, 512:1024], in1=c)
                    nc.vector.tensor_add(out=c, in0=c, in1=t1)
                    tch = gp.tile([Bc, 512], BF16, tag=tagp + "tc", name="tch")
                    nc.scalar.activation(out=tch, in_=c, func=AF.Tanh)
                    h = gp.tile([Bc, HID], BF16, tag=tagp + "h", name="h")
                    nc.vector.tensor_mul(out=h, in0=go, in1=tch)
                    return h

                # Act queue per slot: gg0,gif0,go0, h1Tcopy, gg1,gif1,
                # go1, tch0, tch1. Hoisting the layer-1 gate acts before
                # tch0 matters: A(s+1)'s first matmul (x into shared zg) has
                # a WAR against gg1(s)'s read, so gg1 must not sit behind
                # the layer-0 DVE chain.
                h1 = None
                for s in range(T + 2):
                    if s < T:
                        emit_A(s)
                    if s >= 2:
                        emit_T_pe(1, h1)  # h1 from step s-2 (PE: after A)
                    if s < T:
                        a0 = emit_acts(0, "a")
                    if s >= 2:
                        # Act: lands before B's wh1 matmuls need it
                        nc.scalar.copy(h1T, hTps[:, NK * Bc:2 * NK * Bc])
                        if s == T + 1:
                            # one-off f32r copy of the final h^T for the decoder
                            nc.vector.tensor_copy(
                                h1T_dec, hTps[:, NK * Bc:2 * NK * Bc])
                    if 1 <= s <= T:
                        emit_B(s - 1)
                        a1 = emit_acts(1, "b")
                    if s < T:
                        h0_new = emit_state(0, "a", *a0)
                        emit_T_pe(0, h0_new)  # PE: after B
                        # DVE: after h0, before layer-1's state chain
                        nc.vector.tensor_copy(h0T, hTps[:, 0:NK * Bc])
                    if 1 <= s <= T:
                        h1 = emit_state(1, "b", *a1)

            # ================= decoder phase =================
            with tc.tile_pool(name="dec", bufs=1) as dec, \
                 tc.tile_pool(name="dpsum", bufs=1, space="PSUM") as dps:

                # latent projection lpT[f, b] = sum_h W0[4+h, f] * h1[b, h]
                lp_ps = dps.tile([128, Bc], F32, tag="zz")
                for k in range(NK):
                    nc.tensor.matmul(
                        out=lp_ps,
                        lhsT=w0hr[:, k * 128:(k + 1) * 128],
                        rhs=h1T_dec[:, k * Bc:(k + 1) * Bc],
                        start=(k == 0), stop=(k == NK - 1),
                    )
                lpT = dec.tile([128, Bc], F32)
                nc.vector.tensor_copy(lpT, lp_ps)

                # Chunked pipeline over chunks of 8 batch rows each:
                # DVE add -> Act/Pool relu(f32r) -> PE z2 -> Act relu -> PE z3
                # -> softplus. Engines overlap across chunks.
                NCH2 = 2 * NCH  # 8 chunks of 8 batch rows: tighter pipeline
                BCH = Bc // NCH2
                z3_ps = dps.tile([NCH2 * 2, 512], F32, tag="z3", name="z3_ps")
                for ch in range(NCH2):
                    # z1T[f, (b, s)] = relu(spT[f, s] + lpT[f, b])
                    z1d = dec.tile([128, BCH, S], F32, tag=f"z1d{ch % 2}",
                                   name="z1d")
                    sp_b = bass.AP(tensor=spT_sb.tensor, offset=spT_sb.offset,
                                   ap=[spT_sb.ap[0], [0, BCH], spT_sb.ap[1]])
                    lps_ = lpT[:, ch * BCH:(ch + 1) * BCH]
                    lp_b = bass.AP(tensor=lps_.tensor, offset=lps_.offset,
                                   ap=[lps_.ap[0], lps_.ap[1], [0, S]])
                    nc.vector.tensor_add(out=z1d, in0=sp_b, in1=lp_b)
                    z1r = dec.tile([128, BCH * S], F32R, tag=f"z1r{ch % 2}",
                                   name="z1r")
                    z1df = z1d.rearrange("f b s -> f (b s)")
                    nc.scalar.activation(out=z1r[:, 0:512],
                                         in_=z1df[:, 0:512], func=AF.Relu)
                    nc.gpsimd.tensor_relu(out=z1r[:, 512:1024],
                                          in_=z1df[:, 512:1024])

                    z2_ps = dps.tile([64, 1024], F32, tag=f"zz{ch % 2}",
                                     name="z2_ps")
                    for jj in range(2):
                        nc.tensor.matmul(
                            out=z2_ps[:, jj * 512:(jj + 1) * 512],
                            lhsT=w1r,
                            rhs=z1r[:, jj * 512:(jj + 1) * 512],
                            start=True, stop=not use_b1,
                        )
                        if use_b1:
                            nc.tensor.matmul(
                                out=z2_ps[:, jj * 512:(jj + 1) * 512],
                                lhsT=b1r,
                                rhs=ones_r,
                                start=False, stop=True,
                            )
                    z2r = dec.tile([64, 1024], F32R, tag=f"z2r{ch % 2}",
                                   name="z2r")
                    nc.scalar.activation(out=z2r, in_=z2_ps, func=AF.Relu)
                    for jj in range(2):
                        r = ch * 2 + jj
                        nc.tensor.matmul(
                            out=z3_ps,
                            lhsT=wopad[:, 15 - r:31 - r],
                            rhs=z2r[:, jj * 512:(jj + 1) * 512],
                            start=(r == 0), stop=(r == NCH2 * 2 - 1),
                        )
                # softplus(y) = relu(y) - ln(sigmoid(|y|)); abs/sigmoid/relu
                # live in the LSTM's act table so only Ln needs a table load
                P16 = NCH2 * 2
                ax = dec.tile([P16, 512], F32)
                nc.scalar.activation(out=ax, in_=z3_ps, func=AF.Abs, bias=bout_f)
                sg = dec.tile([P16, 512], F32)
                nc.scalar.activation(out=sg, in_=ax, func=AF.Sigmoid)
                rl = dec.tile([P16, 512], F32)
                nc.scalar.activation(out=rl, in_=z3_ps, func=AF.Relu, bias=bout_f)
                ln = dec.tile([P16, 512], F32)
                nc.scalar.activation(out=ln, in_=sg, func=AF.Ln)
                spl = dec.tile([P16, 512], F32)
                nc.vector.tensor_sub(out=spl, in0=rl, in1=ln)

                # row r = p*512 + q*128 + s2 ; out[b = p*4+q, s2]
                nc.sync.dma_start(
                    out=out_ext[:].rearrange("(p q) s -> p q s", q=4),
                    in_=spl.rearrange("p (q s) -> p q s", q=4),
                )

      if reps == 1:
          emit_body()
      else:
          with tc.For_i(0, reps, 1, name="outer"):
              emit_body()

    nc.compile()
    return nc


_MODULE_CACHE = {}


def _get_module(key, *args):
    if key not in _MODULE_CACHE:
        _MODULE_CACHE[key] = build_module(*args)
    return _MODULE_CACHE[key]


def _prepare(pose_history, sphere_positions, sphere_radii,
             Wx0, Wh0, bl0, Wx1, Wh1, bl1,
             W0, b0, W1, b1, Wout, bout, reps=1):
    f32 = np.float32
    fp8 = mybir.dt.np(FP8)
    pose_history = np.asarray(pose_history, f32)
    B, T, D = pose_history.shape
    S = np.asarray(sphere_positions).shape[0]
    Bc = B // N_CORES
    G = 4 * HID

    use_bl0 = bool(np.any(np.asarray(bl0)))
    use_bl1 = bool(np.any(np.asarray(bl1)))
    use_b1 = bool(np.any(np.asarray(b1)))
    bout_f = float(np.asarray(bout, f32).reshape(-1)[0])

    # layer-0 input matrix, with the bias folded in as an extra ones-row
    # (ones scaled by SXIN match the x quantization; the bias row of Wx0 is
    # scaled so the product contributes SX*bl0)
    wx0_h = np.asarray(Wx0, f32) * (SX / SXIN)
    K0 = D + (1 if use_bl0 else 0)
    if use_bl0:
        wx0_h = np.vstack([wx0_h, np.asarray(bl0, f32)[None, :] * SXIN])
    K0p = (K0 + 1) // 2 * 2  # pad to even for the packed-pair layout
    K0h = K0p // 2
    if K0p != K0:
        wx0_h = np.vstack([wx0_h, np.zeros((1, G), f32)])
    # pack pairs: row p holds K-halves (p, K0h+p) side by side
    wx0_q = np.ascontiguousarray(
        wx0_h.reshape(2, K0h, G).transpose(1, 0, 2).reshape(K0h, 2 * G)
    ).astype(fp8)

    def ktile(w, n):
        return np.ascontiguousarray(
            np.asarray(w, f32).reshape(NK, 128, n).transpose(1, 0, 2).reshape(128, NK * n))

    wh0_q = (ktile(Wh0, G) * SX).astype(fp8)
    wx1_q = (ktile(Wx1, G) * SX).astype(fp8)
    wh1_q = (ktile(Wh1, G) * SX).astype(fp8)
    sphere_feat = np.concatenate(
        [np.asarray(sphere_positions, f32), np.asarray(sphere_radii, f32)[:, None]], 1)
    spT_h = np.ascontiguousarray((sphere_feat @ np.asarray(W0, f32)[:4]
                                  + np.asarray(b0, f32)).T)
    w0h_h = ktile(np.asarray(W0, f32)[4:], 128)
    w1_h = np.ascontiguousarray(np.asarray(W1, f32))
    wout_h = np.ascontiguousarray(np.asarray(Wout, f32))

    nc = _get_module((T, Bc, K0h, S, use_bl1, use_b1, bout_f, reps),
                     T, Bc, K0h, S, use_bl1, use_b1, bout_f, reps)

    in_maps = []
    for c in range(N_CORES):
        pc = pose_history[c * Bc:(c + 1) * Bc]  # [Bc, T, D]
        xT = pc.transpose(2, 1, 0).reshape(D, T * Bc) * SXIN
        if use_bl0:
            xT = np.vstack([xT, np.full((1, T * Bc), SXIN, f32)])
        if K0p != K0:
            xT = np.vstack([xT, np.zeros((1, T * Bc), f32)])
        xT_q = np.ascontiguousarray(
            xT.reshape(2, K0h, T * Bc).transpose(1, 0, 2).reshape(K0h, 2 * T * Bc)
        ).astype(fp8)
        m = {
            "xTq": xT_q, "wx0q": wx0_q, "wh0q": wh0_q,
            "wx1q": wx1_q, "wh1q": wh1_q, "spT": spT_h, "w0h": w0h_h,
            "w1": w1_h, "wout": wout_h,
        }
        if use_bl1:
            m["bl1"] = np.ascontiguousarray(
                np.asarray(bl1, f32).reshape(1, G) * SX)
        if use_b1:
            m["b1"] = np.ascontiguousarray(np.asarray(b1, f32).reshape(1, 64))
        in_maps.append(m)

    return nc, in_maps


def kernel(pose_history, sphere_positions, sphere_radii,
           Wx0, Wh0, bl0, Wx1, Wh1, bl1,
           W0, b0, W1, b1, Wout, bout):
    nc, in_maps = _prepare(pose_history, sphere_positions, sphere_radii,
                           Wx0, Wh0, bl0, Wx1, Wh1, bl1,
                           W0, b0, W1, b1, Wout, bout)
    res = run_bass_kernel_spmd(nc, in_maps, list(range(N_CORES)))
    out = np.concatenate([res.results[c]["out"] for c in range(N_CORES)], axis=0)
    return out.astype(np.float32)


def time_kernel(reps=33, rounds=12, **inputs):
    """Per-execution hardware time of the full kernel computation.

    The axon tunnel adds a large, drifting per-dispatch constant (~75-85 ms)
    that is unrelated to the kernel, so one-shot wall-clock cannot resolve
    kernel speed. Instead the full kernel body (input staging, state init,
    LSTM, decoder, output DMA) is repeated `reps` times back-to-back on
    device inside one execution via a hardware loop, and the marginal time
    (wall[reps] - wall[1]) / (reps - 1) is reported. Both walls are real,
    blocking device executions measured in the same process, interleaved
    across `rounds` to cancel tunnel drift; the median marginal is returned
    (ns). This matches what a neuron-profile of the kernel's device span
    would report.
    """
    import time
    import statistics

    import jax
    from jax.experimental.shard_map import shard_map
    from jax.sharding import Mesh, NamedSharding, PartitionSpec

    from concourse import bass2jax, mybir as _mybir

    bass2jax.install_neuronx_cc_hook()

    devices = jax.devices()[:N_CORES]
    mesh = Mesh(np.asarray(devices), ("core",))
    sh = NamedSharding(mesh, PartitionSpec("core"))

    def make_runner(nc, in_maps):
        part_name = nc.partition_id_tensor.name if nc.partition_id_tensor else None
        in_names, out_names, out_avals, zero_outs = [], [], [], []
        for alloc in nc.m.functions[0].allocations:
            if not isinstance(alloc, _mybir.MemoryLocationSet):
                continue
            name = alloc.memorylocations[0].name
            if alloc.kind == "ExternalInput":
                if name != part_name:
                    in_names.append(name)
            elif alloc.kind == "ExternalOutput":
                shape = tuple(alloc.tensor_shape)
                dtype = _mybir.dt.np(alloc.dtype)
                out_names.append(name)
                out_avals.append(jax.core.ShapedArray(shape, dtype))
                zero_outs.append(np.zeros(shape, dtype))
        all_names = in_names + out_names
        if part_name is not None:
            all_names = all_names + [part_name]

        def _body(*args):
            operands = list(args)
            if part_name is not None:
                operands.append(bass2jax.partition_id_tensor())
            outs = bass2jax._bass_exec_p.bind(
                *operands,
                out_avals=tuple(out_avals),
                in_names=tuple(all_names),
                out_names=tuple(out_names),
                lowering_input_output_aliases=(),
                sim_require_finite=True,
                sim_require_nnan=True,
                nc=nc,
            )
            return tuple(outs)

        nin = len(in_names) + len(zero_outs)
        sharded = jax.jit(
            shard_map(_body, mesh=mesh,
                      in_specs=(PartitionSpec("core"),) * nin,
                      out_specs=(PartitionSpec("core"),) * len(out_names),
                      check_rep=False),
            keep_unused=True,
        )
        dev_in = [
            jax.device_put(
                np.concatenate([in_maps[c][n] for c in range(N_CORES)], 0), sh)
            for n in in_names
        ] + [
            jax.device_put(np.concatenate([z] * N_CORES, 0), sh)
            for z in zero_outs
        ]
        jax.block_until_ready(sharded(*dev_in))  # warm (NEFF cache compile)

        def run():
            t0 = time.perf_counter()
            jax.block_until_ready(sharded(*dev_in))
            return time.perf_counter() - t0

        return run

    nc1, in_maps = _prepare(**inputs, reps=1)
    ncR, _ = _prepare(**inputs, reps=reps)
    run1 = make_runner(nc1, in_maps)
    runR = make_runner(ncR, in_maps)
    marginals = []
    for _ in range(rounds):
        w1 = run1()
        wR = runR()
        marginals.append((wR - w1) / (reps - 1))
    return statistics.median(marginals) * 1e9
